# revision 6
# baseline (speedup 1.0000x reference)
"""DCT spatial interaction kernel for Trainium2 (8 NeuronCores, SPMD).

Computes out = x * g for x [16, 96, 224, 224] f32, where (matching the
reference's einsum orientation, whose "inverse" transforms are in fact a
second forward application):

    F = D X D^T,  g = D (mask * F) D^T

with D the 224-point orthonormal DCT-II matrix and mask zeroing the 56x56
top-left block. Since mask*F = F - E F E (E selects indices < 56):

    g = R X R^T - G (A X A^T) G^T
    R = D @ D,  A = D[:56, :],  G = D[:, :56]

Sharding: batch dim split 2-per-core across 8 cores; 192 slices per core.

Per-slice dataflow (pair = 2 slices batched where operands are shared;
group = 4 pairs for the Ft stage):
  V  = X^T R^T            4 f32r matmuls, X-chunks as STATIONARY operand
                          (free transpose), rhs = R column-permuted, N=256
  Y  = (V^T-chunks) R^T   4 f32r matmuls into g-PSUM, N=256
  T1 = A X                2 f32r matmuls, N=448/pair
  T1t = T1^T              4 small fp32 PE transposes / pair
  Ft = A T1^T (=F_ll^T)   2 f32r matmuls per 8 slices, N=448
  W2 = F_ll G^T           1 f32r matmul / slice, N=256
  g -= G W2               2 f32r matmuls / slice accumulated into g-PSUM
                          with negated G constants  ->  g = Y - Z in PSUM
  out = x * g             1 DVE multiply / slice, DMA out.

h rows are interleaved 2-per-partition ([112, 448] slice tiles, 1792B DMA
lines); all h-side constants are column-permuted (even/odd) to match.
"""

from contextlib import ExitStack

import numpy as np

import concourse.bass as bass
import concourse.tile as tile
from concourse import bacc, mybir
from concourse.bass_utils import run_bass_kernel_spmd

F32 = mybir.dt.float32
F32R = mybir.dt.float32r

N_CORES = 8
B, C, H, W = 16, 96, 224, 224
SLICES = (B // N_CORES) * C       # 192 per core
PAIRS = SLICES // 2               # 96
GROUPS = PAIRS // 4               # 24
P = 112                           # partitions per slice tile (2 h-rows each)


def _dct(n):
    k = np.arange(n, dtype=np.float64)[:, None]
    i = np.arange(n, dtype=np.float64)[None, :]
    m = np.cos(np.pi * (2.0 * i + 1.0) * k / (2.0 * n)) * np.sqrt(2.0 / n)
    m[0] *= 1.0 / np.sqrt(2.0)
    return m


def _consts():
    D = _dct(224)
    A = D[:56, :]                                     # [56, 224]
    R = D @ D                                         # [224, 224]
    G = D[:, :56]                                     # [224, 56]
    At = A.T                                          # [224, 56]

    def f32(a):
        return np.ascontiguousarray(a, dtype=np.float32)

    def padc(a, n):
        return np.pad(a, ((0, 0), (0, n - a.shape[1])))

    return {
        "c_at_e": f32(At[0::2]),                      # [112, 56]
        "c_at_o": f32(At[1::2]),                      # [112, 56]
        "c_bt0": f32(At[:128]),                       # [128, 56]
        "c_bt1": f32(At[128:]),                       # [96, 56]
        "c_gt": f32(padc(G.T, 256)),                  # [56, 256]
        "c_ge": f32(-G[0::2, :].T),                   # [56, 112]
        "c_go": f32(-G[1::2, :].T),                   # [56, 112]
        "c_rpe": f32(padc(R[:, 0::2].T, 256)),        # [112, 256]
        "c_rpo": f32(padc(R[:, 1::2].T, 256)),        # [112, 256]
        "c_rt0": f32(padc(R.T[0:128], 256)),          # [128, 256]
        "c_rt1": f32(padc(R.T[128:224], 256)),        # [96, 256]
        "c_ident": np.eye(128, dtype=np.float32),     # [128, 128]
    }


_CONST_SPECS = [
    ("c_at_e", [112, 56], F32R), ("c_at_o", [112, 56], F32R),
    ("c_bt0", [128, 56], F32R), ("c_bt1", [96, 56], F32R),
    ("c_gt", [56, 256], F32R), ("c_ge", [56, 112], F32R),
    ("c_go", [56, 112], F32R), ("c_rpe", [112, 256], F32R),
    ("c_rpo", [112, 256], F32R), ("c_rt0", [128, 256], F32R),
    ("c_rt1", [96, 256], F32R), ("c_ident", [128, 128], F32),
]


def build_nc(n_slices=SLICES):
    n_groups = n_slices // 8
    assert n_groups * 8 == n_slices

    nc = bacc.Bacc("TRN2", target_bir_lowering=False, debug=False)
    x = nc.dram_tensor("x", [n_slices, P, 448], F32R, kind="ExternalInput")
    out = nc.dram_tensor("out", [n_slices, P, 448], F32, kind="ExternalOutput")
    cdram = {n: nc.dram_tensor(n, s, d, kind="ExternalInput")
             for n, s, d in _CONST_SPECS}

    with tile.TileContext(nc) as tc, ExitStack() as ctx:
        cpool = ctx.enter_context(tc.tile_pool(name="consts", bufs=1))
        xpool = ctx.enter_context(tc.tile_pool(name="x", bufs=4))
        rpool = ctx.enter_context(tc.tile_pool(name="res", bufs=4))
        vspool = ctx.enter_context(tc.tile_pool(name="vs", bufs=3))
        t1spool = ctx.enter_context(tc.tile_pool(name="t1s", bufs=2))
        tgpool = ctx.enter_context(tc.tile_pool(name="t1tg", bufs=2))
        ftspool = ctx.enter_context(tc.tile_pool(name="fts", bufs=2))
        w2spool = ctx.enter_context(tc.tile_pool(name="w2s", bufs=2))

        ps_v = ctx.enter_context(tc.tile_pool(name="ps_v", bufs=2, space="PSUM"))
        ps_g = ctx.enter_context(tc.tile_pool(name="ps_g", bufs=2, space="PSUM"))
        ps_t1 = ctx.enter_context(tc.tile_pool(name="ps_t1", bufs=1, space="PSUM"))
        ps_tt = ctx.enter_context(tc.tile_pool(name="ps_tt", bufs=1, space="PSUM"))
        ps_ft = ctx.enter_context(tc.tile_pool(name="ps_ft", bufs=1, space="PSUM"))
        ps_w2 = ctx.enter_context(tc.tile_pool(name="ps_w2", bufs=1, space="PSUM"))

        ct = {}
        for n, s, d in _CONST_SPECS:
            ct[n] = cpool.tile(s, d, name=n)
            nc.sync.dma_start(ct[n][:], cdram[n].ap())

        for g in range(n_groups):
            t1tg = tgpool.tile([128, 896], F32R)   # (pair, s, chunk, k)
            pair_xt = []
            pair_vs = []
            for jj in range(4):
                pr = 4 * g + jj
                xt = xpool.tile([112, 896], F32R)  # (s, e, w)
                nc.sync.dma_start(xt[:, 0:448], x.ap()[2 * pr])
                nc.sync.dma_start(xt[:, 448:896], x.ap()[2 * pr + 1])
                xt4 = xt[:].rearrange("p (s e w) -> p s e w", s=2, e=2, w=224)

                # V = X^T R^T per slice: [128, 512] = [w-chunk | r(pad)]
                vss = []
                for s in range(2):
                    vp = ps_v.tile([128, 512], F32)
                    for c, (w0, w1) in enumerate(((0, 128), (128, 224))):
                        for e in range(2):
                            nc.tensor.matmul(
                                vp[0:w1 - w0, c * 256:(c + 1) * 256],
                                xt4[:, s:s + 1, e:e + 1, w0:w1],
                                ct["c_rpe" if e == 0 else "c_rpo"][:],
                                start=(e == 0), stop=(e == 1))
                    vs = vspool.tile([128, 512], F32R)
                    nc.vector.tensor_copy(vs[:], vp[:])
                    vss.append(vs)
                pair_vs.append(vss)

                # T1 = A @ X  [56, (s, w)]
                t1 = ps_t1.tile([56, 448], F32)
                nc.tensor.matmul(t1[:], ct["c_at_e"][:], xt4[:, :, 0:1, :],
                                 start=True, stop=False)
                nc.tensor.matmul(t1[:], ct["c_at_o"][:], xt4[:, :, 1:2, :],
                                 start=False, stop=True)
                t1s = t1spool.tile([56, 448], F32)
                nc.scalar.copy(t1s[:], t1[:])

                # T1t packed per pair as [128, (s, chunk, k)]
                tt = ps_tt.tile([128, 224], F32)
                for s in range(2):
                    nc.tensor.transpose(
                        tt[0:128, s * 112: s * 112 + 56],
                        t1s[:, s * 224: s * 224 + 128],
                        ct["c_ident"][0:56, 0:56])
                    nc.tensor.transpose(
                        tt[0:96, s * 112 + 56: s * 112 + 112],
                        t1s[:, s * 224 + 128: s * 224 + 224],
                        ct["c_ident"][0:56, 0:56])
                nc.scalar.copy(t1tg[:, jj * 224:(jj + 1) * 224], tt[:])
                pair_xt.append(xt)

            # Ft = A @ T1^T for 8 slices  [56, (pair, s, k)]
            tg5 = t1tg[:].rearrange("p (q s c k) -> p q s c k",
                                    q=4, s=2, c=2, k=56)
            ft = ps_ft.tile([56, 448], F32)
            nc.tensor.matmul(ft[:], ct["c_bt0"][:], tg5[:, :, :, 0:1, :],
                             start=True, stop=False)
            nc.tensor.matmul(ft[:], ct["c_bt1"][:], tg5[0:96, :, :, 1:2, :],
                             start=False, stop=True)
            fts = ftspool.tile([56, 448], F32R)
            nc.scalar.copy(fts[:], ft[:])

            w2s_list = []
            for jj in range(4):
                # W2 = F_ll @ G^T(pad) per slice; pair packed [56, 512]
                w2 = ps_w2.tile([56, 512], F32)
                for s in range(2):
                    i = 2 * jj + s
                    nc.tensor.matmul(w2[:, s * 256:(s + 1) * 256],
                                     fts[:, i * 56:(i + 1) * 56], ct["c_gt"][:],
                                     start=True, stop=True)
                w2s = w2spool.tile([56, 512], F32R)
                nc.scalar.copy(w2s[:], w2[:])
                w2s_list.append(w2s)

            for jj in range(4):
                pr = 4 * g + jj
                xt = pair_xt[jj]
                xt4 = xt[:].rearrange("p (s e w) -> p s e w", s=2, e=2, w=224)
                res = rpool.tile([112, 896], F32)
                res4 = res[:].rearrange("p (s e w) -> p s e w", s=2, e=2, w=224)

                for s in range(2):
                    vs = pair_vs[jj][s]
                    vsr = vs[:].rearrange("p (c z) -> p c z", c=2, z=256)
                    vsr = vsr[:, :, 0:224].rearrange("p c (q e) -> p c e q",
                                                     q=112, e=2)
                    gp = ps_g.tile([112, 512], F32)
                    for a in range(2):       # output h parity
                        reg = gp[:, a * 256:(a + 1) * 256]
                        nc.tensor.matmul(reg, vsr[:, 0:1, a:a + 1, :],
                                         ct["c_rt0"][:],
                                         start=True, stop=False)
                        nc.tensor.matmul(reg, vsr[0:96, 1:2, a:a + 1, :],
                                         ct["c_rt1"][:],
                                         start=False, stop=False)
                        # g -= G @ W2  (negated constants, accumulate)
                        nc.tensor.matmul(reg,
                                         ct["c_ge" if a == 0 else "c_go"][:],
                                         w2s_list[jj][:, s * 256:(s + 1) * 256],
                                         start=False, stop=True)
                    # out = x * g
                    gv = gp[:].rearrange("p (a w) -> p a w", a=2, w=256)
                    nc.vector.tensor_mul(
                        res4[:, s:s + 1, :, :],
                        xt4[:, s:s + 1, :, :].bitcast(F32),
                        gv[:, :, 0:224])
                    nc.sync.dma_start(out.ap()[2 * pr + s],
                                      res[:, s * 448:(s + 1) * 448])

    nc.compile()
    return nc


_NC_CACHE = {}


def _get_nc(n_slices=SLICES):
    if n_slices not in _NC_CACHE:
        _NC_CACHE[n_slices] = build_nc(n_slices)
    return _NC_CACHE[n_slices]


def kernel(x: np.ndarray) -> np.ndarray:
    assert x.shape == (B, C, H, W) and x.dtype == np.float32
    nc = _get_nc()
    consts = _consts()
    shards = np.ascontiguousarray(x).reshape(N_CORES, SLICES, P, 448)
    in_maps = [dict(consts, x=shards[i]) for i in range(N_CORES)]
    res = run_bass_kernel_spmd(nc, in_maps, core_ids=list(range(N_CORES)))
    outa = np.stack([res.results[i]["out"] for i in range(N_CORES)])
    return outa.reshape(B, C, H, W)


# revision 7
# speedup vs baseline: 1.0351x; 1.0351x over previous
"""DCT spatial interaction kernel for Trainium2 (8 NeuronCores, SPMD).

Computes out = x * g for x [16, 96, 224, 224] f32, where (matching the
reference's einsum orientation, whose "inverse" transforms are in fact a
second forward application):

    F = D X D^T,  g = D (mask * F) D^T

with D the 224-point orthonormal DCT-II matrix and mask zeroing the 56x56
top-left block. Since mask*F = F - E F E (E selects indices < 56):

    g = R X R^T - G (A X A^T) G^T
    R = D @ D,  A = D[:56, :],  G = D[:, :56]

Sharding: batch dim split 2-per-core across 8 cores; 192 slices per core.

Per-slice dataflow (pair = 2 slices batched where operands are shared;
group = 4 pairs for the Ft stage):
  V  = X^T R^T            4 f32r matmuls, X-chunks as STATIONARY operand
                          (free transpose), rhs = R column-permuted, N=256
  Y  = (V^T-chunks) R^T   4 f32r matmuls into g-PSUM, N=256
  T1 = A X                2 f32r matmuls, N=448/pair
  T1t = T1^T              4 small fp32 PE transposes / pair
  Ft = A T1^T (=F_ll^T)   2 f32r matmuls per 8 slices, N=448
  W2 = F_ll G^T           1 f32r matmul / slice, N=256
  g -= G W2               2 f32r matmuls / slice accumulated into g-PSUM
                          with negated G constants  ->  g = Y - Z in PSUM
  out = x * g             1 DVE multiply / slice, DMA out.

h rows are interleaved 2-per-partition ([112, 448] slice tiles, 1792B DMA
lines); all h-side constants are column-permuted (even/odd) to match.
"""

from contextlib import ExitStack

import numpy as np

import concourse.bass as bass
import concourse.tile as tile
from concourse import bacc, mybir
from concourse.bass_utils import run_bass_kernel_spmd

F32 = mybir.dt.float32
F32R = mybir.dt.float32r
BF16 = mybir.dt.bfloat16

N_CORES = 8
B, C, H, W = 16, 96, 224, 224
SLICES = (B // N_CORES) * C       # 192 per core
PAIRS = SLICES // 2               # 96
GROUPS = PAIRS // 4               # 24
P = 112                           # partitions per slice tile (2 h-rows each)


def _dct(n):
    k = np.arange(n, dtype=np.float64)[:, None]
    i = np.arange(n, dtype=np.float64)[None, :]
    m = np.cos(np.pi * (2.0 * i + 1.0) * k / (2.0 * n)) * np.sqrt(2.0 / n)
    m[0] *= 1.0 / np.sqrt(2.0)
    return m


def _consts():
    D = _dct(224)
    A = D[:56, :]                                     # [56, 224]
    R = D @ D                                         # [224, 224]
    G = D[:, :56]                                     # [224, 56]
    At = A.T                                          # [224, 56]

    def f32(a):
        return np.ascontiguousarray(a, dtype=np.float32)

    def bf16(a):
        import ml_dtypes
        return np.ascontiguousarray(a, dtype=ml_dtypes.bfloat16)

    def padc(a, n):
        return np.pad(a, ((0, 0), (0, n - a.shape[1])))

    return {
        "c_at_e": f32(At[0::2]),                      # [112, 56]
        "c_at_o": f32(At[1::2]),                      # [112, 56]
        "c_bt0": f32(At[:128]),                       # [128, 56]
        "c_bt1": f32(At[128:]),                       # [96, 56]
        "c_gt": f32(padc(G.T, 256)),                  # [56, 256]
        "c_ge": f32(-G[0::2, :].T),                   # [56, 112]
        "c_go": f32(-G[1::2, :].T),                   # [56, 112]
        "c_rpe": bf16(R[:, 0::2].T),                  # [112, 224]
        "c_rpo": bf16(R[:, 1::2].T),                  # [112, 224]
        "c_rt0": bf16(R.T[0:128]),                    # [128, 224]
        "c_rt1": bf16(R.T[128:224]),                  # [96, 224]
        "c_ident": np.eye(128, dtype=np.float32),     # [128, 128]
    }


_CONST_SPECS = [
    ("c_at_e", [112, 56], F32R), ("c_at_o", [112, 56], F32R),
    ("c_bt0", [128, 56], F32R), ("c_bt1", [96, 56], F32R),
    ("c_gt", [56, 256], F32R), ("c_ge", [56, 112], F32R),
    ("c_go", [56, 112], F32R), ("c_rpe", [112, 224], BF16),
    ("c_rpo", [112, 224], BF16), ("c_rt0", [128, 224], BF16),
    ("c_rt1", [96, 224], BF16), ("c_ident", [128, 128], F32),
]


def build_nc(n_slices=SLICES):
    n_groups = n_slices // 8
    assert n_groups * 8 == n_slices

    nc = bacc.Bacc("TRN2", target_bir_lowering=False, debug=False)
    x = nc.dram_tensor("x", [n_slices, P, 448], F32R, kind="ExternalInput")
    out = nc.dram_tensor("out", [n_slices, P, 448], F32, kind="ExternalOutput")
    cdram = {n: nc.dram_tensor(n, s, d, kind="ExternalInput")
             for n, s, d in _CONST_SPECS}

    with tile.TileContext(nc) as tc, ExitStack() as ctx:
        cpool = ctx.enter_context(tc.tile_pool(name="consts", bufs=1))
        xpool = ctx.enter_context(tc.tile_pool(name="x", bufs=4))
        xbpool = ctx.enter_context(tc.tile_pool(name="xb", bufs=3))
        rpool = ctx.enter_context(tc.tile_pool(name="res", bufs=4))
        vspool = ctx.enter_context(tc.tile_pool(name="vs", bufs=3))
        t1spool = ctx.enter_context(tc.tile_pool(name="t1s", bufs=2))
        tgpool = ctx.enter_context(tc.tile_pool(name="t1tg", bufs=2))
        ftspool = ctx.enter_context(tc.tile_pool(name="fts", bufs=2))
        w2spool = ctx.enter_context(tc.tile_pool(name="w2s", bufs=2))

        ps_v = ctx.enter_context(tc.tile_pool(name="ps_v", bufs=2, space="PSUM"))
        ps_g = ctx.enter_context(tc.tile_pool(name="ps_g", bufs=2, space="PSUM"))
        ps_t1 = ctx.enter_context(tc.tile_pool(name="ps_t1", bufs=1, space="PSUM"))
        ps_tt = ctx.enter_context(tc.tile_pool(name="ps_tt", bufs=1, space="PSUM"))
        ps_ft = ctx.enter_context(tc.tile_pool(name="ps_ft", bufs=1, space="PSUM"))
        ps_w2 = ctx.enter_context(tc.tile_pool(name="ps_w2", bufs=1, space="PSUM"))

        ct = {}
        for n, s, d in _CONST_SPECS:
            ct[n] = cpool.tile(s, d, name=n)
            nc.sync.dma_start(ct[n][:], cdram[n].ap())

        for g in range(n_groups):
            t1tg = tgpool.tile([128, 896], F32R)   # (pair, s, chunk, k)
            pair_xt = []
            pair_vs = []
            for jj in range(4):
                pr = 4 * g + jj
                xt = xpool.tile([112, 896], F32R)  # (s, e, w)
                nc.sync.dma_start(xt[:, 0:448], x.ap()[2 * pr])
                nc.sync.dma_start(xt[:, 448:896], x.ap()[2 * pr + 1])
                xt4 = xt[:].rearrange("p (s e w) -> p s e w", s=2, e=2, w=224)
                xb = xbpool.tile([112, 896], BF16)
                nc.gpsimd.tensor_copy(xb[:], xt[:].bitcast(F32))
                xb4 = xb[:].rearrange("p (s e w) -> p s e w", s=2, e=2, w=224)

                # V = X^T R^T per slice (bf16): [128, 448] = [w-chunk, r]
                vss = []
                for s in range(2):
                    vp = ps_v.tile([128, 448], F32)
                    for c, (w0, w1) in enumerate(((0, 128), (128, 224))):
                        for e in range(2):
                            nc.tensor.matmul(
                                vp[0:w1 - w0, c * 224:(c + 1) * 224],
                                xb4[:, s:s + 1, e:e + 1, w0:w1],
                                ct["c_rpe" if e == 0 else "c_rpo"][:],
                                start=(e == 0), stop=(e == 1))
                    vs = vspool.tile([128, 448], BF16)
                    nc.vector.tensor_copy(vs[:], vp[:])
                    vss.append(vs)
                pair_vs.append(vss)

                # T1 = A @ X  [56, (s, w)]
                t1 = ps_t1.tile([56, 448], F32)
                nc.tensor.matmul(t1[:], ct["c_at_e"][:], xt4[:, :, 0:1, :],
                                 start=True, stop=False)
                nc.tensor.matmul(t1[:], ct["c_at_o"][:], xt4[:, :, 1:2, :],
                                 start=False, stop=True)
                t1s = t1spool.tile([56, 448], F32)
                nc.scalar.copy(t1s[:], t1[:])

                # T1t packed per pair as [128, (s, chunk, k)]
                tt = ps_tt.tile([128, 224], F32)
                for s in range(2):
                    nc.tensor.transpose(
                        tt[0:128, s * 112: s * 112 + 56],
                        t1s[:, s * 224: s * 224 + 128],
                        ct["c_ident"][0:56, 0:56])
                    nc.tensor.transpose(
                        tt[0:96, s * 112 + 56: s * 112 + 112],
                        t1s[:, s * 224 + 128: s * 224 + 224],
                        ct["c_ident"][0:56, 0:56])
                nc.scalar.copy(t1tg[:, jj * 224:(jj + 1) * 224], tt[:])
                pair_xt.append(xt)

            # Ft = A @ T1^T for 8 slices  [56, (pair, s, k)]
            tg5 = t1tg[:].rearrange("p (q s c k) -> p q s c k",
                                    q=4, s=2, c=2, k=56)
            ft = ps_ft.tile([56, 448], F32)
            nc.tensor.matmul(ft[:], ct["c_bt0"][:], tg5[:, :, :, 0:1, :],
                             start=True, stop=False)
            nc.tensor.matmul(ft[:], ct["c_bt1"][:], tg5[0:96, :, :, 1:2, :],
                             start=False, stop=True)
            fts = ftspool.tile([56, 448], F32R)
            nc.scalar.copy(fts[:], ft[:])

            w2s_list = []
            for jj in range(4):
                # W2 = F_ll @ G^T(pad) per slice; pair packed [56, 512]
                w2 = ps_w2.tile([56, 512], F32)
                for s in range(2):
                    i = 2 * jj + s
                    nc.tensor.matmul(w2[:, s * 256:(s + 1) * 256],
                                     fts[:, i * 56:(i + 1) * 56], ct["c_gt"][:],
                                     start=True, stop=True)
                w2s = w2spool.tile([56, 512], F32R)
                nc.scalar.copy(w2s[:], w2[:])
                w2s_list.append(w2s)

            for jj in range(4):
                pr = 4 * g + jj
                xt = pair_xt[jj]
                xt4 = xt[:].rearrange("p (s e w) -> p s e w", s=2, e=2, w=224)
                res = rpool.tile([112, 896], F32)
                res4 = res[:].rearrange("p (s e w) -> p s e w", s=2, e=2, w=224)

                for s in range(2):
                    vs = pair_vs[jj][s]
                    vsr = vs[:].rearrange("p (c q e) -> p c e q",
                                          c=2, q=112, e=2)
                    gp = ps_g.tile([112, 512], F32)
                    for a in range(2):       # output h parity
                        nc.tensor.matmul(gp[:, a * 256:a * 256 + 224],
                                         vsr[:, 0:1, a:a + 1, :],
                                         ct["c_rt0"][:],
                                         start=True, stop=False,
                                         skip_group_check=True)
                        nc.tensor.matmul(gp[:, a * 256:a * 256 + 224],
                                         vsr[0:96, 1:2, a:a + 1, :],
                                         ct["c_rt1"][:],
                                         start=False, stop=False,
                                         skip_group_check=True)
                        # g -= G @ W2  (negated constants, accumulate)
                        nc.tensor.matmul(gp[:, a * 256:(a + 1) * 256],
                                         ct["c_ge" if a == 0 else "c_go"][:],
                                         w2s_list[jj][:, s * 256:(s + 1) * 256],
                                         start=False, stop=True,
                                         skip_group_check=True)
                    # out = x * g
                    gv = gp[:].rearrange("p (a w) -> p a w", a=2, w=256)
                    nc.vector.tensor_mul(
                        res4[:, s:s + 1, :, :],
                        xt4[:, s:s + 1, :, :].bitcast(F32),
                        gv[:, :, 0:224])
                    nc.sync.dma_start(out.ap()[2 * pr + s],
                                      res[:, s * 448:(s + 1) * 448])

    nc.compile()
    return nc


_NC_CACHE = {}


def _get_nc(n_slices=SLICES):
    if n_slices not in _NC_CACHE:
        _NC_CACHE[n_slices] = build_nc(n_slices)
    return _NC_CACHE[n_slices]


def kernel(x: np.ndarray) -> np.ndarray:
    assert x.shape == (B, C, H, W) and x.dtype == np.float32
    nc = _get_nc()
    consts = _consts()
    shards = np.ascontiguousarray(x).reshape(N_CORES, SLICES, P, 448)
    in_maps = [dict(consts, x=shards[i]) for i in range(N_CORES)]
    res = run_bass_kernel_spmd(nc, in_maps, core_ids=list(range(N_CORES)))
    outa = np.stack([res.results[i]["out"] for i in range(N_CORES)])
    return outa.reshape(B, C, H, W)


# revision 8
# speedup vs baseline: 1.0395x; 1.0043x over previous
"""DCT spatial interaction kernel for Trainium2 (8 NeuronCores, SPMD).

Computes out = x * g for x [16, 96, 224, 224] f32, where (matching the
reference's einsum orientation, whose "inverse" transforms are in fact a
second forward application):

    F = D X D^T,  g = D (mask * F) D^T

with D the 224-point orthonormal DCT-II matrix and mask zeroing the 56x56
top-left block. Since mask*F = F - E F E (E selects indices < 56):

    g = R X R^T - G (A X A^T) G^T
    R = D @ D,  A = D[:56, :],  G = D[:, :56]

Sharding: batch dim split 2-per-core across 8 cores; 192 slices per core.

Per-slice dataflow (pair = 2 slices batched where operands are shared;
group = 4 pairs for the Ft stage):
  V  = X^T R^T            4 f32r matmuls, X-chunks as STATIONARY operand
                          (free transpose), rhs = R column-permuted, N=256
  Y  = (V^T-chunks) R^T   4 f32r matmuls into g-PSUM, N=256
  T1 = A X                2 f32r matmuls, N=448/pair
  T1t = T1^T              4 small fp32 PE transposes / pair
  Ft = A T1^T (=F_ll^T)   2 f32r matmuls per 8 slices, N=448
  W2 = F_ll G^T           1 f32r matmul / slice, N=256
  g -= G W2               2 f32r matmuls / slice accumulated into g-PSUM
                          with negated G constants  ->  g = Y - Z in PSUM
  out = x * g             1 DVE multiply / slice, DMA out.

h rows are interleaved 2-per-partition ([112, 448] slice tiles, 1792B DMA
lines); all h-side constants are column-permuted (even/odd) to match.
"""

from contextlib import ExitStack

import numpy as np

import concourse.bass as bass
import concourse.tile as tile
from concourse import bacc, mybir
from concourse.bass_utils import run_bass_kernel_spmd

F32 = mybir.dt.float32
F32R = mybir.dt.float32r
BF16 = mybir.dt.bfloat16

N_CORES = 8
B, C, H, W = 16, 96, 224, 224
SLICES = (B // N_CORES) * C       # 192 per core
PAIRS = SLICES // 2               # 96
GROUPS = PAIRS // 4               # 24
P = 112                           # partitions per slice tile (2 h-rows each)


def _dct(n):
    k = np.arange(n, dtype=np.float64)[:, None]
    i = np.arange(n, dtype=np.float64)[None, :]
    m = np.cos(np.pi * (2.0 * i + 1.0) * k / (2.0 * n)) * np.sqrt(2.0 / n)
    m[0] *= 1.0 / np.sqrt(2.0)
    return m


def _consts():
    D = _dct(224)
    A = D[:56, :]                                     # [56, 224]
    R = D @ D                                         # [224, 224]
    G = D[:, :56]                                     # [224, 56]
    At = A.T                                          # [224, 56]

    def f32(a):
        return np.ascontiguousarray(a, dtype=np.float32)

    def bf16(a):
        import ml_dtypes
        return np.ascontiguousarray(a, dtype=ml_dtypes.bfloat16)

    def padc(a, n):
        return np.pad(a, ((0, 0), (0, n - a.shape[1])))

    return {
        "c_at_e": f32(At[0::2]),                      # [112, 56]
        "c_at_o": f32(At[1::2]),                      # [112, 56]
        "c_bt0": f32(At[:128]),                       # [128, 56]
        "c_bt1": f32(At[128:]),                       # [96, 56]
        "c_gt": f32(padc(G.T, 256)),                  # [56, 256]
        "c_ge": f32(-G[0::2, :].T),                   # [56, 112]
        "c_go": f32(-G[1::2, :].T),                   # [56, 112]
        "c_rpe": bf16(R[:, 0::2].T),                  # [112, 224]
        "c_rpo": bf16(R[:, 1::2].T),                  # [112, 224]
        "c_rt0": bf16(R.T[0:128]),                    # [128, 224]
        "c_rt1": bf16(R.T[128:224]),                  # [96, 224]
        "c_ident": np.eye(128, dtype=np.float32),     # [128, 128]
    }


_CONST_SPECS = [
    ("c_at_e", [112, 56], F32R), ("c_at_o", [112, 56], F32R),
    ("c_bt0", [128, 56], F32R), ("c_bt1", [96, 56], F32R),
    ("c_gt", [56, 256], F32R), ("c_ge", [56, 112], F32R),
    ("c_go", [56, 112], F32R), ("c_rpe", [112, 224], BF16),
    ("c_rpo", [112, 224], BF16), ("c_rt0", [128, 224], BF16),
    ("c_rt1", [96, 224], BF16), ("c_ident", [128, 128], F32),
]


def build_nc(n_slices=SLICES):
    n_groups = n_slices // 8
    assert n_groups * 8 == n_slices

    nc = bacc.Bacc("TRN2", target_bir_lowering=False, debug=False)
    x = nc.dram_tensor("x", [n_slices, P, 448], F32R, kind="ExternalInput")
    out = nc.dram_tensor("out", [n_slices, P, 448], F32, kind="ExternalOutput")
    cdram = {n: nc.dram_tensor(n, s, d, kind="ExternalInput")
             for n, s, d in _CONST_SPECS}

    with tile.TileContext(nc) as tc, ExitStack() as ctx:
        cpool = ctx.enter_context(tc.tile_pool(name="consts", bufs=1))
        xpool = ctx.enter_context(tc.tile_pool(name="x", bufs=6))
        xbpool = ctx.enter_context(tc.tile_pool(name="xb", bufs=3))
        rpool = ctx.enter_context(tc.tile_pool(name="res", bufs=4))
        vspool = ctx.enter_context(tc.tile_pool(name="vs", bufs=10))
        t1spool = ctx.enter_context(tc.tile_pool(name="t1s", bufs=3))
        tgpool = ctx.enter_context(tc.tile_pool(name="t1tg", bufs=2))
        ftspool = ctx.enter_context(tc.tile_pool(name="fts", bufs=2))
        w2spool = ctx.enter_context(tc.tile_pool(name="w2s", bufs=6))

        ps_v = ctx.enter_context(tc.tile_pool(name="ps_v", bufs=2, space="PSUM"))
        ps_g = ctx.enter_context(tc.tile_pool(name="ps_g", bufs=2, space="PSUM"))
        ps_t1 = ctx.enter_context(tc.tile_pool(name="ps_t1", bufs=1, space="PSUM"))
        ps_tt = ctx.enter_context(tc.tile_pool(name="ps_tt", bufs=1, space="PSUM"))
        ps_ft = ctx.enter_context(tc.tile_pool(name="ps_ft", bufs=1, space="PSUM"))
        ps_w2 = ctx.enter_context(tc.tile_pool(name="ps_w2", bufs=1, space="PSUM"))

        ct = {}
        for n, s, d in _CONST_SPECS:
            ct[n] = cpool.tile(s, d, name=n)
            nc.sync.dma_start(ct[n][:], cdram[n].ap())

        for g in range(n_groups):
            t1tg = tgpool.tile([128, 896], F32R)   # (pair, s, chunk, k)
            pair_xt = []
            pair_vs = []
            for jj in range(4):
                pr = 4 * g + jj
                xt = xpool.tile([112, 896], F32R)  # (s, e, w)
                nc.sync.dma_start(xt[:, 0:448], x.ap()[2 * pr])
                nc.sync.dma_start(xt[:, 448:896], x.ap()[2 * pr + 1])
                xt4 = xt[:].rearrange("p (s e w) -> p s e w", s=2, e=2, w=224)
                xb = xbpool.tile([112, 896], BF16)
                nc.gpsimd.tensor_copy(xb[:], xt[:].bitcast(F32))
                xb4 = xb[:].rearrange("p (s e w) -> p s e w", s=2, e=2, w=224)

                # V = X^T R^T per slice (bf16): [128, 448] = [w-chunk, r]
                vss = []
                for s in range(2):
                    vp = ps_v.tile([128, 448], F32)
                    for c, (w0, w1) in enumerate(((0, 128), (128, 224))):
                        for e in range(2):
                            nc.tensor.matmul(
                                vp[0:w1 - w0, c * 224:(c + 1) * 224],
                                xb4[:, s:s + 1, e:e + 1, w0:w1],
                                ct["c_rpe" if e == 0 else "c_rpo"][:],
                                start=(e == 0), stop=(e == 1))
                    vs = vspool.tile([128, 448], BF16)
                    nc.vector.tensor_copy(vs[:], vp[:])
                    vss.append(vs)
                pair_vs.append(vss)

                # T1 = A @ X  [56, (s, w)]
                t1 = ps_t1.tile([56, 448], F32)
                nc.tensor.matmul(t1[:], ct["c_at_e"][:], xt4[:, :, 0:1, :],
                                 start=True, stop=False)
                nc.tensor.matmul(t1[:], ct["c_at_o"][:], xt4[:, :, 1:2, :],
                                 start=False, stop=True)
                t1s = t1spool.tile([56, 448], F32)
                nc.scalar.copy(t1s[:], t1[:])

                # T1t packed per pair as [128, (s, chunk, k)]
                tt = ps_tt.tile([128, 224], F32)
                for s in range(2):
                    nc.tensor.transpose(
                        tt[0:128, s * 112: s * 112 + 56],
                        t1s[:, s * 224: s * 224 + 128],
                        ct["c_ident"][0:56, 0:56])
                    nc.tensor.transpose(
                        tt[0:96, s * 112 + 56: s * 112 + 112],
                        t1s[:, s * 224 + 128: s * 224 + 224],
                        ct["c_ident"][0:56, 0:56])
                nc.scalar.copy(t1tg[:, jj * 224:(jj + 1) * 224], tt[:])
                pair_xt.append(xt)

            # Ft = A @ T1^T for 8 slices  [56, (pair, s, k)]
            tg5 = t1tg[:].rearrange("p (q s c k) -> p q s c k",
                                    q=4, s=2, c=2, k=56)
            ft = ps_ft.tile([56, 448], F32)
            nc.tensor.matmul(ft[:], ct["c_bt0"][:], tg5[:, :, :, 0:1, :],
                             start=True, stop=False)
            nc.tensor.matmul(ft[:], ct["c_bt1"][:], tg5[0:96, :, :, 1:2, :],
                             start=False, stop=True)
            fts = ftspool.tile([56, 448], F32R)
            nc.scalar.copy(fts[:], ft[:])

            w2s_list = []
            for jj in range(4):
                # W2 = F_ll @ G^T(pad) per slice; pair packed [56, 512]
                w2 = ps_w2.tile([56, 512], F32)
                for s in range(2):
                    i = 2 * jj + s
                    nc.tensor.matmul(w2[:, s * 256:(s + 1) * 256],
                                     fts[:, i * 56:(i + 1) * 56], ct["c_gt"][:],
                                     start=True, stop=True)
                w2s = w2spool.tile([56, 512], F32R)
                nc.scalar.copy(w2s[:], w2[:])
                w2s_list.append(w2s)

            for jj in range(4):
                pr = 4 * g + jj
                xt = pair_xt[jj]
                xt4 = xt[:].rearrange("p (s e w) -> p s e w", s=2, e=2, w=224)
                res = rpool.tile([112, 896], F32)
                res4 = res[:].rearrange("p (s e w) -> p s e w", s=2, e=2, w=224)

                for s in range(2):
                    vs = pair_vs[jj][s]
                    vsr = vs[:].rearrange("p (c q e) -> p c e q",
                                          c=2, q=112, e=2)
                    gp = ps_g.tile([112, 512], F32)
                    for a in range(2):       # output h parity
                        nc.tensor.matmul(gp[:, a * 256:a * 256 + 224],
                                         vsr[:, 0:1, a:a + 1, :],
                                         ct["c_rt0"][:],
                                         start=True, stop=False,
                                         skip_group_check=True)
                        nc.tensor.matmul(gp[:, a * 256:a * 256 + 224],
                                         vsr[0:96, 1:2, a:a + 1, :],
                                         ct["c_rt1"][:],
                                         start=False, stop=False,
                                         skip_group_check=True)
                        # g -= G @ W2  (negated constants, accumulate)
                        nc.tensor.matmul(gp[:, a * 256:(a + 1) * 256],
                                         ct["c_ge" if a == 0 else "c_go"][:],
                                         w2s_list[jj][:, s * 256:(s + 1) * 256],
                                         start=False, stop=True,
                                         skip_group_check=True)
                    # out = x * g
                    gv = gp[:].rearrange("p (a w) -> p a w", a=2, w=256)
                    nc.vector.tensor_mul(
                        res4[:, s:s + 1, :, :],
                        xt4[:, s:s + 1, :, :].bitcast(F32),
                        gv[:, :, 0:224])
                    nc.sync.dma_start(out.ap()[2 * pr + s],
                                      res[:, s * 448:(s + 1) * 448])

    nc.compile()
    return nc


_NC_CACHE = {}


def _get_nc(n_slices=SLICES):
    if n_slices not in _NC_CACHE:
        _NC_CACHE[n_slices] = build_nc(n_slices)
    return _NC_CACHE[n_slices]


def kernel(x: np.ndarray) -> np.ndarray:
    assert x.shape == (B, C, H, W) and x.dtype == np.float32
    nc = _get_nc()
    consts = _consts()
    shards = np.ascontiguousarray(x).reshape(N_CORES, SLICES, P, 448)
    in_maps = [dict(consts, x=shards[i]) for i in range(N_CORES)]
    res = run_bass_kernel_spmd(nc, in_maps, core_ids=list(range(N_CORES)))
    outa = np.stack([res.results[i]["out"] for i in range(N_CORES)])
    return outa.reshape(B, C, H, W)


# revision 9
# speedup vs baseline: 1.3018x; 1.2523x over previous
"""DCT spatial interaction kernel for Trainium2 (8 NeuronCores, SPMD).

Computes out = x * g for x [16, 96, 224, 224] f32, where (matching the
reference's einsum orientation, whose "inverse" transforms are in fact a
second forward application):

    F = D X D^T,  g = D (mask * F) D^T

with D the 224-point orthonormal DCT-II matrix and mask zeroing the 56x56
top-left block. Since mask*F = F - E F E (E selects indices < 56):

    g = R X R^T - G (A X A^T) G^T
    R = D @ D,  A = D[:56, :],  G = D[:, :56]

Sharding: batch dim split 2-per-core across 8 cores; 192 slices per core.

Per-slice dataflow (pair = 2 slices batched where operands are shared;
group = 4 pairs for the Ft stage):
  V  = X^T R^T            4 f32r matmuls, X-chunks as STATIONARY operand
                          (free transpose), rhs = R column-permuted, N=256
  Y  = (V^T-chunks) R^T   4 f32r matmuls into g-PSUM, N=256
  T1 = A X                2 f32r matmuls, N=448/pair
  T1t = T1^T              4 small fp32 PE transposes / pair
  Ft = A T1^T (=F_ll^T)   2 f32r matmuls per 8 slices, N=448
  W2 = F_ll G^T           1 f32r matmul / slice, N=256
  g -= G W2               2 f32r matmuls / slice accumulated into g-PSUM
                          with negated G constants  ->  g = Y - Z in PSUM
  out = x * g             1 DVE multiply / slice, DMA out.

h rows are interleaved 2-per-partition ([112, 448] slice tiles, 1792B DMA
lines); all h-side constants are column-permuted (even/odd) to match.
"""

from contextlib import ExitStack

import numpy as np

import concourse.bass as bass
import concourse.tile as tile
from concourse import bacc, mybir
from concourse.bass_utils import run_bass_kernel_spmd

F32 = mybir.dt.float32
F32R = mybir.dt.float32r
BF16 = mybir.dt.bfloat16

N_CORES = 8
B, C, H, W = 16, 96, 224, 224
SLICES = (B // N_CORES) * C       # 192 per core
PAIRS = SLICES // 2               # 96
GROUPS = PAIRS // 4               # 24
P = 112                           # partitions per slice tile (2 h-rows each)


def _dct(n):
    k = np.arange(n, dtype=np.float64)[:, None]
    i = np.arange(n, dtype=np.float64)[None, :]
    m = np.cos(np.pi * (2.0 * i + 1.0) * k / (2.0 * n)) * np.sqrt(2.0 / n)
    m[0] *= 1.0 / np.sqrt(2.0)
    return m


def _consts():
    D = _dct(224)
    A = D[:56, :]                                     # [56, 224]
    R = D @ D                                         # [224, 224]
    G = D[:, :56]                                     # [224, 56]
    At = A.T                                          # [224, 56]

    def f32(a):
        return np.ascontiguousarray(a, dtype=np.float32)

    def bf16(a):
        import ml_dtypes
        return np.ascontiguousarray(a, dtype=ml_dtypes.bfloat16)

    def padc(a, n):
        return np.pad(a, ((0, 0), (0, n - a.shape[1])))

    return {
        "c_at_e": f32(At[0::2]),                      # [112, 56]
        "c_at_o": f32(At[1::2]),                      # [112, 56]
        "c_bt0": f32(At[:128]),                       # [128, 56]
        "c_bt1": f32(At[128:]),                       # [96, 56]
        "c_gt": f32(padc(G.T, 256)),                  # [56, 256]
        "c_ge": f32(-G[0::2, :].T),                   # [56, 112]
        "c_go": f32(-G[1::2, :].T),                   # [56, 112]
        "c_rpe": f32(padc(R[:, 0::2].T, 256)),        # [112, 256]
        "c_rpo": f32(padc(R[:, 1::2].T, 256)),        # [112, 256]
        "c_rt0": bf16(R.T[0:128]),                    # [128, 224]
        "c_rt1": bf16(R.T[128:224]),                  # [96, 224]
        "c_ident": np.eye(128, dtype=np.float32),     # [128, 128]
    }


_CONST_SPECS = [
    ("c_at_e", [112, 56], F32R), ("c_at_o", [112, 56], F32R),
    ("c_bt0", [128, 56], F32R), ("c_bt1", [96, 56], F32R),
    ("c_gt", [56, 256], F32R), ("c_ge", [56, 112], F32R),
    ("c_go", [56, 112], F32R), ("c_rpe", [112, 256], F32R),
    ("c_rpo", [112, 256], F32R), ("c_rt0", [128, 224], BF16),
    ("c_rt1", [96, 224], BF16), ("c_ident", [128, 128], F32),
]


def build_nc(n_slices=SLICES):
    n_groups = n_slices // 8
    assert n_groups * 8 == n_slices

    nc = bacc.Bacc("TRN2", target_bir_lowering=False, debug=False)
    x = nc.dram_tensor("x", [n_slices, P, 448], F32R, kind="ExternalInput")
    out = nc.dram_tensor("out", [n_slices, P, 448], F32, kind="ExternalOutput")
    cdram = {n: nc.dram_tensor(n, s, d, kind="ExternalInput")
             for n, s, d in _CONST_SPECS}

    with tile.TileContext(nc) as tc, ExitStack() as ctx:
        cpool = ctx.enter_context(tc.tile_pool(name="consts", bufs=1))
        xpool = ctx.enter_context(tc.tile_pool(name="x", bufs=8))
        rpool = ctx.enter_context(tc.tile_pool(name="res", bufs=4))
        vspool = ctx.enter_context(tc.tile_pool(name="vs", bufs=10))
        t1spool = ctx.enter_context(tc.tile_pool(name="t1s", bufs=3))
        tgpool = ctx.enter_context(tc.tile_pool(name="t1tg", bufs=2))
        ftspool = ctx.enter_context(tc.tile_pool(name="fts", bufs=2))
        w2spool = ctx.enter_context(tc.tile_pool(name="w2s", bufs=6))

        ps_v = ctx.enter_context(tc.tile_pool(name="ps_v", bufs=2, space="PSUM"))
        ps_g = ctx.enter_context(tc.tile_pool(name="ps_g", bufs=2, space="PSUM"))
        ps_t1 = ctx.enter_context(tc.tile_pool(name="ps_t1", bufs=1, space="PSUM"))
        ps_tt = ctx.enter_context(tc.tile_pool(name="ps_tt", bufs=1, space="PSUM"))
        ps_ft = ctx.enter_context(tc.tile_pool(name="ps_ft", bufs=1, space="PSUM"))
        ps_w2 = ctx.enter_context(tc.tile_pool(name="ps_w2", bufs=1, space="PSUM"))

        ct = {}
        for n, s, d in _CONST_SPECS:
            ct[n] = cpool.tile(s, d, name=n)
            nc.sync.dma_start(ct[n][:], cdram[n].ap())

        for g in range(n_groups):
            t1tg = tgpool.tile([128, 896], F32R)   # (pair, s, chunk, k)
            pair_xt = []
            pair_vs = []
            for jj in range(4):
                pr = 4 * g + jj
                xt = xpool.tile([112, 896], F32R)  # (s, e, w)
                nc.sync.dma_start(xt[:, 0:448], x.ap()[2 * pr])
                nc.sync.dma_start(xt[:, 448:896], x.ap()[2 * pr + 1])
                xt4 = xt[:].rearrange("p (s e w) -> p s e w", s=2, e=2, w=224)
                # V = X^T R^T per slice (f32r): psum [128, 512]
                vss = []
                for s in range(2):
                    vp = ps_v.tile([128, 512], F32)
                    for c, (w0, w1) in enumerate(((0, 128), (128, 224))):
                        for e in range(2):
                            nc.tensor.matmul(
                                vp[0:w1 - w0, c * 256:(c + 1) * 256],
                                xt4[:, s:s + 1, e:e + 1, w0:w1],
                                ct["c_rpe" if e == 0 else "c_rpo"][:],
                                start=(e == 0), stop=(e == 1))
                    vs = vspool.tile([128, 448], BF16)
                    vpv = vp[:].rearrange("p (c z) -> p c z", c=2, z=256)
                    nc.vector.tensor_copy(vs[:], vpv[:, :, 0:224])
                    vss.append(vs)
                pair_vs.append(vss)

                # T1 = A @ X  [56, (s, w)]
                t1 = ps_t1.tile([56, 448], F32)
                nc.tensor.matmul(t1[:], ct["c_at_e"][:], xt4[:, :, 0:1, :],
                                 start=True, stop=False)
                nc.tensor.matmul(t1[:], ct["c_at_o"][:], xt4[:, :, 1:2, :],
                                 start=False, stop=True)
                t1s = t1spool.tile([56, 448], F32)
                nc.scalar.copy(t1s[:], t1[:])

                # T1t packed per pair as [128, (s, chunk, k)]
                tt = ps_tt.tile([128, 224], F32)
                for s in range(2):
                    nc.tensor.transpose(
                        tt[0:128, s * 112: s * 112 + 56],
                        t1s[:, s * 224: s * 224 + 128],
                        ct["c_ident"][0:56, 0:56])
                    nc.tensor.transpose(
                        tt[0:96, s * 112 + 56: s * 112 + 112],
                        t1s[:, s * 224 + 128: s * 224 + 224],
                        ct["c_ident"][0:56, 0:56])
                nc.scalar.copy(t1tg[:, jj * 224:(jj + 1) * 224], tt[:])
                pair_xt.append(xt)

            # Ft = A @ T1^T for 8 slices  [56, (pair, s, k)]
            tg5 = t1tg[:].rearrange("p (q s c k) -> p q s c k",
                                    q=4, s=2, c=2, k=56)
            ft = ps_ft.tile([56, 448], F32)
            nc.tensor.matmul(ft[:], ct["c_bt0"][:], tg5[:, :, :, 0:1, :],
                             start=True, stop=False)
            nc.tensor.matmul(ft[:], ct["c_bt1"][:], tg5[0:96, :, :, 1:2, :],
                             start=False, stop=True)
            fts = ftspool.tile([56, 448], F32R)
            nc.scalar.copy(fts[:], ft[:])

            w2s_list = []
            for jj in range(4):
                # W2 = F_ll @ G^T(pad) per slice; pair packed [56, 512]
                w2 = ps_w2.tile([56, 512], F32)
                for s in range(2):
                    i = 2 * jj + s
                    nc.tensor.matmul(w2[:, s * 256:(s + 1) * 256],
                                     fts[:, i * 56:(i + 1) * 56], ct["c_gt"][:],
                                     start=True, stop=True)
                w2s = w2spool.tile([56, 512], F32R)
                nc.scalar.copy(w2s[:], w2[:])
                w2s_list.append(w2s)

            for jj in range(4):
                pr = 4 * g + jj
                xt = pair_xt[jj]
                xt4 = xt[:].rearrange("p (s e w) -> p s e w", s=2, e=2, w=224)
                res = rpool.tile([112, 896], F32)
                res4 = res[:].rearrange("p (s e w) -> p s e w", s=2, e=2, w=224)

                for s in range(2):
                    vs = pair_vs[jj][s]
                    vsr = vs[:].rearrange("p (c q e) -> p c e q",
                                          c=2, q=112, e=2)
                    gp = ps_g.tile([112, 512], F32)
                    for a in range(2):       # output h parity
                        nc.tensor.matmul(gp[:, a * 256:a * 256 + 224],
                                         vsr[:, 0:1, a:a + 1, :],
                                         ct["c_rt0"][:],
                                         start=True, stop=False,
                                         skip_group_check=True)
                        nc.tensor.matmul(gp[:, a * 256:a * 256 + 224],
                                         vsr[0:96, 1:2, a:a + 1, :],
                                         ct["c_rt1"][:],
                                         start=False, stop=False,
                                         skip_group_check=True)
                        # g -= G @ W2  (negated constants, accumulate)
                        nc.tensor.matmul(gp[:, a * 256:(a + 1) * 256],
                                         ct["c_ge" if a == 0 else "c_go"][:],
                                         w2s_list[jj][:, s * 256:(s + 1) * 256],
                                         start=False, stop=True,
                                         skip_group_check=True)
                    # out = x * g
                    gv = gp[:].rearrange("p (a w) -> p a w", a=2, w=256)
                    nc.vector.tensor_mul(
                        res4[:, s:s + 1, :, :],
                        xt4[:, s:s + 1, :, :].bitcast(F32),
                        gv[:, :, 0:224])
                    nc.sync.dma_start(out.ap()[2 * pr + s],
                                      res[:, s * 448:(s + 1) * 448])

    nc.compile()
    return nc


_NC_CACHE = {}


def _get_nc(n_slices=SLICES):
    if n_slices not in _NC_CACHE:
        _NC_CACHE[n_slices] = build_nc(n_slices)
    return _NC_CACHE[n_slices]


def kernel(x: np.ndarray) -> np.ndarray:
    assert x.shape == (B, C, H, W) and x.dtype == np.float32
    nc = _get_nc()
    consts = _consts()
    shards = np.ascontiguousarray(x).reshape(N_CORES, SLICES, P, 448)
    in_maps = [dict(consts, x=shards[i]) for i in range(N_CORES)]
    res = run_bass_kernel_spmd(nc, in_maps, core_ids=list(range(N_CORES)))
    outa = np.stack([res.results[i]["out"] for i in range(N_CORES)])
    return outa.reshape(B, C, H, W)
